# revision 32
# baseline (speedup 1.0000x reference)
"""AttentionPool kernel for 8x Trainium2 NeuronCores (Bass/Tile).

Problem (per batch b of B=8):
    q = (x[:, :8] @ Wq.T).reshape(8, 24, 64) * 64**-0.5
    k = (x @ Wk.T).reshape(4096, 24, 64)
    v = (x @ Wv.T).reshape(4096, 24, 64)
    attn = softmax(mask(q @ k.T))          # [24, 8, 4096]
    out = (attn @ v).reshape(8, 1536) @ Wp.T + bp

Sharding: data-parallel over B - one batch per NeuronCore, no collectives.

Work split: the device does ALL the O(N) token work; the host does only
O(R*C^2) weight folding (R=8):
  device  logits[n, hr] = x[n, :] . q2[hr, :]   (q2 = q*scale @ Wk folded
          on host), masked exp -> eT, unnormalized pool
          p[c, hr] = sum_n x[n, c] e[n, hr], partial denominators.
  host    q2 fold (76 MFLOP), then the weight epilogue in exact fp32:
          x_cls[r, hb] = (p[:, h*8+r]/den) @ Wv.T[:, hb], out = x_cls @ Wp.T
          + bp (0.9 GFLOP of the 312 GFLOP problem). Wv/Wp never ship to
          the device at all.

Precision/DMA budget (per core, 360 GB/s DMA model, 2.4 GHz PE):
  xt   e4m3  6.29 MB  logits GEMM in DoubleRow perf mode (2 c-tiles per
                      instruction at 0.5 cyc/row): 30.7us -> 15.4us even
                      with the q2 hi/lo residual double-pass.
  q2   e4m3 hi+lo, scaled by 256 (avoids e4m3 subnormal flush at
                      |q2|~0.02), descaled inside the Act exp (scale=1/256).
                      The lo pass cancels the q2 quantization error.
  xn   e3m4  6.29 MB  pool GEMM in normal mode, f16 moving eT (e4m3 xn
                      would cost 1.6e-2 of the 2e-2 budget vs 0.8e-2).
  out  slab [128, 12, 192] f16 + den128 [128, 192] f16 (0.64 MB).
  Host-emulated end-to-end rel err: 1.63e-2 (gate 2e-2, deterministic).

  DMA total 13.9 MB ~= 38.5us; PE = 15.4 (logits DR) + 30.7 (pool) ~= 46us
  is the critical path. All host arrays are partition-major so every
  transfer is >=512B contiguous runs. Stream order: xt0a, q2hi, q2lo,
  xt0b, mask, then xt one chunk AHEAD of xn (xt1, xn0, xt2, xn1, ...) so
  the in-order PE (logits nt, then pool nt-1) never waits on the stream.

Schedule:
  a memset-fed warmup matmul anchors the PE p-state ramp at ~0.5us so all
  real matmuls are charged at full clock (ramp model: 3us to 2.4 GHz).
  per 512-token chunk: logits per 128-token subtile (12 DoubleRow matmuls:
      6 ct-pairs x {q2hi, q2lo}) -> exp (Act, psum -> f32, scale 1/256) ->
      * mask (DVE, broadcast over heads) -> fp16 eT. Pool and den of the
      PREVIOUS chunk are emitted after this chunk's logits so the PE never
      stalls on the exp->mask chain: per c-tile psum[c128, 192] over 4
      subtiles (stationary = e3m4 x subtile, moving = eT), drained into
      one fp16 slab; den[1, 192] via DVE reduce.
  tail: den DMA fires as soon as den(7) lands; the pool-7 slab ships in
      two 6-c-tile pieces (second via the idle Act queue) so the last
      piece's issue overlaps the first piece's transfer.
"""

import numpy as np
import ml_dtypes

B, N, C = 8, 4096, 1536
H, HD, R = 24, 64, 8
HR = H * R           # 192 (h, r) pairs, index hr = h*R + r
SCALE = HD ** -0.5
P = 128
CT = C // P          # 12 contraction tiles
CTP = CT // 2        # 6 DoubleRow ct-pairs
NCHUNK = 512
NSUB_CH = NCHUNK // P  # 4 subtiles per chunk
NT = N // NCHUNK     # 8 chunks
NSUB = N // P        # 32 token subtiles total

Q2SCALE = 256.0      # q2 pre-scale (e4m3 subnormal avoidance)

_RUNNER_CACHE = {}


def _build():
    import concourse.mybir as mybir
    import concourse.tile as tile
    from concourse import bacc

    F32 = mybir.dt.float32
    F16 = mybir.dt.float16
    F8E3 = mybir.dt.float8e3
    F8E4 = mybir.dt.float8e4
    MULT = mybir.AluOpType.mult
    SUB = mybir.AluOpType.subtract
    ADD = mybir.AluOpType.add
    EXP = mybir.ActivationFunctionType.Exp
    DR = mybir.MatmulPerfMode.DoubleRow

    nc = bacc.Bacc(None, target_bir_lowering=False)
    # x.T partition-major: [p, nt, half, ct, 256] = x[512nt+256h+n', 128ct+p]
    xt = nc.dram_tensor("xt", [P, NT, 2, CT, NCHUNK // 2], F8E4,
                        kind="ExternalInput")
    # x partition-major: [p, nt, s, c] = x[512nt+128s+p, c]; chunks 0-3
    # ship e3m4 (pool in normal mode), chunks 4-7 e4m3 (pool in DoubleRow
    # with hi/lo e4m3 eT)
    xn3 = nc.dram_tensor("xn3", [P, NT // 2, NSUB_CH, C], F8E3,
                         kind="ExternalInput")
    xn4 = nc.dram_tensor("xn4", [P, NT // 2, NSUB_CH, C], F8E4,
                         kind="ExternalInput")
    q2hi = nc.dram_tensor("q2hi", [P, CT, HR], F8E4, kind="ExternalInput")
    q2lo = nc.dram_tensor("q2lo", [P, CT, HR], F8E4, kind="ExternalInput")
    # mask partition-major: [p, s, r] = mask_full[r, 128s+p]
    maskt = nc.dram_tensor("maskt", [P, NSUB, R], F16, kind="ExternalInput")
    slabout = nc.dram_tensor("slabout", [P, CT, HR], F16,
                             kind="ExternalOutput")
    denout = nc.dram_tensor("denout", [P, HR], F16, kind="ExternalOutput")

    with tile.TileContext(nc) as tc:
        with (
            tc.tile_pool(name="pper", bufs=1) as pper,      # persistent
            tc.tile_pool(name="pxt", bufs=4) as pxt,        # xT chunks
            tc.tile_pool(name="pxn", bufs=4) as pxn,
            tc.tile_pool(name="pxn4", bufs=4) as pxn4,        # x chunks
            tc.tile_pool(name="pexp", bufs=2) as pexp,
            tc.tile_pool(name="ptmp", bufs=3) as ptmp,
            tc.tile_pool(name="pslab", bufs=1) as pslab,
            tc.tile_pool(name="ps_l", bufs=2, space="PSUM") as ps_l,
            tc.tile_pool(name="ps_p", bufs=4, space="PSUM") as ps_p,
            tc.tile_pool(name="ps_d", bufs=1, space="PSUM") as ps_d,
        ):
            # ---------- persistent tiles ----------
            q2hi_sb = pper.tile([P, CT, HR], F8E4, tag="q2hi")
            q2lo_sb = pper.tile([P, CT, HR], F8E4, tag="q2lo")
            maskt_sb = pper.tile([P, NSUB, R], F16, tag="maskt")
            eT = pper.tile([P, NSUB, HR], F16, tag="eT")        # masked exp
            # e4m3 hi/lo split of eT for the DoubleRow pool chunks (si>=16)
            eH = pper.tile([P, NSUB // 2, HR], F8E4, tag="eH")
            eL = pper.tile([P, NSUB // 2, HR], F8E4, tag="eL")
            den128 = pper.tile([P, HR], F16, tag="den128")
            warm = pper.tile([P, 8], F16, tag="warm")

            # ---------- DMA emission helpers (order == queue order) -------
            # xt chunk tile: [p, half, ct, 256]; subtile s of the chunk is
            # [:, s // 2, :, (s % 2) * P:(s % 2 + 1) * P]
            xt_ch0 = pxt.tile([P, 2, CT, NCHUNK // 2], F8E4, tag="xt")

            def _xt_chunk(xt_ch, nt):
                nc.sync.dma_start(xt_ch, xt[:, nt])

            xn_ch0 = pxn.tile([P, NSUB_CH, C], F8E3, tag="xn")

            def _xn_chunk(xn_ch, nt):
                if nt < NT // 2:
                    nc.sync.dma_start(xn_ch, xn3[:, nt])
                else:
                    nc.sync.dma_start(xn_ch, xn4[:, nt - NT // 2])

            # warmup: memset-fed tiny matmuls anchor the PE p-state ramp at
            # ~0.5us, long before the first data-gated matmul, so all real
            # matmuls run at full clock (the ramp model needs 3us of busy
            # history to reach 2.4 GHz)
            nc.vector.memset(warm, 1.0)
            for w in range(3):
                ps_w = ps_d.tile([P, 512], F32, tag="pd")
                nc.tensor.matmul(
                    ps_w[0:1, 0:1], warm[:, w:w + 1], warm[:, w:w + 1],
                    start=True, stop=True)

            # startup stream, ordered by need-time
            nc.sync.dma_start(xt_ch0[:, 0], xt[:, 0, 0])
            nc.sync.dma_start(q2hi_sb, q2hi[:])
            nc.sync.dma_start(q2lo_sb, q2lo[:])
            nc.sync.dma_start(xt_ch0[:, 1], xt[:, 0, 1])
            nc.sync.dma_start(maskt_sb, maskt[:])

            # ---------- per-chunk pipeline ----------
            def emit_logits(nt, xt_ch):
                for s in range(NSUB_CH):
                    si = nt * NSUB_CH + s
                    ps = ps_l.tile([P, 512], F32, tag="pl")
                    lT = ps[:, 0:HR]
                    xsub = xt_ch[:, s // 2, :, (s % 2) * P:(s % 2 + 1) * P]
                    for qi, q2sb in enumerate((q2hi_sb, q2lo_sb)):
                        for t in range(CTP):
                            nc.tensor.matmul(
                                lT,
                                xsub[:, 2 * t:2 * t + 2],
                                q2sb[:, 2 * t:2 * t + 2],
                                start=(qi == 0 and t == 0),
                                stop=(qi == 1 and t == CTP - 1),
                                perf_mode=DR)
                    # exp descales the q2 pre-scale: e = exp(psum / 256)
                    nc.scalar.activation(eT[:, si], lT, EXP, scale=1.0 / Q2SCALE)
                    # in-place 0/1 mask: all operands fp16+SBUF, so the DVE
                    # runs in 2x mode; product is exact (mask is 0 or 1)
                    nc.vector.tensor_tensor(
                        eT[:, si].rearrange("p (h r) -> p h r", h=H),
                        eT[:, si].rearrange("p (h r) -> p h r", h=H),
                        maskt_sb[:, si, None, :].to_broadcast((P, H, R)),
                        MULT)
                    if si >= NSUB // 2:
                        # hi/lo e4m3 split for the DoubleRow pool: hi on the
                        # Act engine, residual lo = eT - hi on the DVE
                        sj = si - NSUB // 2
                        nc.gpsimd.tensor_copy(eH[:, sj], eT[:, si])
                        nc.vector.tensor_tensor(
                            eL[:, sj], eT[:, si], eH[:, sj], SUB)

            def emit_pool(nt, xn_ch, slab, ct_lo=0, ct_hi=CT):
                # pool psum per c-tile; slab accumulates chunks in fp16
                # (adds cost ~5e-4 relative - fine). First chunk drains as
                # copies split across DVE and Act; later chunks add on DVE.
                dr = nt >= NT // 2
                for ct in range(ct_lo, ct_hi):
                    ps = ps_p.tile([P, 512], F32, tag="pp")
                    pch = ps[:, 0:HR]
                    if dr:
                        # DoubleRow: k-tile pairs are subtile pairs; two
                        # passes (eH, eL) cancel the e4m3 eT quantization
                        sj0 = (nt - NT // 2) * NSUB_CH
                        for ei, esb in enumerate((eH, eL)):
                            for u in range(NSUB_CH // 2):
                                nc.tensor.matmul(
                                    pch,
                                    xn_ch[:, 2 * u:2 * u + 2,
                                          ct * P:(ct + 1) * P],
                                    esb[:, sj0 + 2 * u:sj0 + 2 * u + 2],
                                    start=(ei == 0 and u == 0),
                                    stop=(ei == 1 and u == NSUB_CH // 2 - 1),
                                    perf_mode=DR)
                    else:
                        for s in range(NSUB_CH):
                            si = nt * NSUB_CH + s
                            nc.tensor.matmul(
                                pch,
                                xn_ch[:, s, ct * P:(ct + 1) * P],
                                eT[:, si],
                                start=(s == 0), stop=(s == NSUB_CH - 1))
                    # drains split across engines so the DVE isn't the
                    # pacer: even cts add on DVE; odd cts route PSUM->SBUF
                    # through an Act copy (GPSIMD can't read PSUM) and add
                    # SBUF->SBUF on the idle GPSIMD
                    if nt == 0:
                        if ct % 2 == 0:
                            nc.vector.tensor_copy(slab[:, ct], pch)
                        else:
                            nc.scalar.copy(slab[:, ct], pch)
                    elif ct % 2 == 0:
                        nc.vector.tensor_add(slab[:, ct], slab[:, ct], pch)
                    else:
                        tmp = ptmp.tile([P, HR], F16, tag="ptmp",
                                        name=f"tmp_{nt}_{ct}")
                        nc.scalar.copy(tmp, pch)
                        nc.gpsimd.tensor_add(slab[:, ct], slab[:, ct], tmp)

            def emit_den(nt):
                # per-partition partial denominators on the DVE (idle
                # capacity): innermost-axis reduce on a strided view; the
                # cross-partition sum happens on the host in fp32.
                sl = eT[:, nt * NSUB_CH:(nt + 1) * NSUB_CH].rearrange(
                    "p s h -> p h s")
                with nc.allow_low_precision(reason="den rel err ~5e-4"):
                    if nt == 0:
                        nc.vector.tensor_reduce(
                            den128, sl, mybir.AxisListType.X, ADD)
                    else:
                        dpart = pexp.tile([P, HR], F16, tag="dpart")
                        nc.vector.tensor_reduce(
                            dpart, sl, mybir.AxisListType.X, ADD)
                        nc.vector.tensor_add(den128, den128, dpart)

            slab = pslab.tile([P, CT, HR], F16, tag="slab", name="slab")
            xns = []
            xt_ch = xt_ch0
            for nt in range(NT):
                # stream xt one chunk AHEAD of xn: the PE runs logits(nt)
                # then pool(nt-1), so it needs xt(nt) before xn(nt-1)
                if nt + 1 < NT:
                    xt_nx = pxt.tile([P, 2, CT, NCHUNK // 2], F8E4, tag="xt")
                    _xt_chunk(xt_nx, nt + 1)
                else:
                    xt_nx = None
                if nt >= NT // 2:
                    xn_ch = pxn4.tile([P, NSUB_CH, C], F8E4, tag="xn4",
                                      name=f"xn_ch{nt}")
                    _xn_chunk(xn_ch, nt)
                elif nt > 0:
                    xn_ch = pxn.tile([P, NSUB_CH, C], F8E3, tag="xn",
                                     name=f"xn_ch{nt}")
                    _xn_chunk(xn_ch, nt)
                else:
                    # chunk 0's xn loads in halves so pool-0 (the head of
                    # the PE's steady pipeline) starts a hop earlier
                    xn_ch = xn_ch0
                    nc.sync.dma_start(xn_ch0[:, 0:2], xn3[:, 0, 0:2])
                    nc.sync.dma_start(xn_ch0[:, 2:4], xn3[:, 0, 2:4])
                emit_logits(nt, xt_ch)
                if nt > 0:
                    emit_pool(nt - 1, xns[nt - 1], slab)
                    emit_den(nt - 1)
                xns.append(xn_ch)
                xt_ch = xt_nx

            # ---------- tail ----------
            # den(7) completes on the DVE as soon as eT(7) is masked; its
            # DMA fires while the pool-7 matmuls still run. The slab ships
            # in two 6-c-tile pieces; the second goes through the idle Act
            # queue so both issue chains overlap.
            emit_den(NT - 1)
            nc.sync.dma_start(denout[:], den128)
            emit_pool(NT - 1, xns[NT - 1], slab, 0, 6)
            nc.sync.dma_start(slabout[:, 0:6], slab[:, 0:6])
            emit_pool(NT - 1, xns[NT - 1], slab, 6, CT)
            nc.sync.dma_start(slabout[:, 6:CT], slab[:, 6:CT])

    nc.compile()
    return nc


def _prep_inputs(x, mask, Wq, Wk):
    """Host-side sharding + layout prep. Returns per-core in_maps.

    The 8-token q projection and its fold through Wk (q2 = q*scale @
    Wk[head rows]) happen here: 76 MFLOP of the 312 GFLOP problem."""
    E3 = ml_dtypes.float8_e3m4
    E4 = ml_dtypes.float8_e4m3

    x = np.asarray(x, dtype=np.float32)
    Wq = np.asarray(Wq, np.float32)
    Wk = np.asarray(Wk, np.float32)

    mask = np.asarray(mask)
    mask_full = np.empty((B, R, N), np.float32)
    mask_full[:, :, :R] = np.eye(R, dtype=np.float32)[None]
    mask_full[:, :, R:] = mask.astype(np.float32)

    # q2[b, hr, c] = sum_d q[b, r, h, d]*SCALE * Wk[h*HD+d, c]
    q = np.einsum('brc,dc->brd', x[:, :R], Wq) * SCALE        # [B, R, C]
    q2 = np.einsum('brhd,hdc->bhrc',
                   q.reshape(B, R, H, HD), Wk.reshape(H, HD, C))
    q2 = q2.reshape(B, HR, C)

    in_maps = []
    for b in range(B):
        xb = np.clip(x[b], -15.0, 15.0)
        # xt: [p, nt, half, ct, 256] = x[512nt+256h+n', 128ct+p], e4m3
        xt_b = np.ascontiguousarray(
            xb.T.reshape(CT, P, NT, 2, NCHUNK // 2)
            .transpose(1, 2, 3, 0, 4).astype(E4))
        # xn: [p, nt, s, c] = x[512nt+128s+p, c]; chunks 0-3 e3m4
        # (normal-mode pool), chunks 4-7 e4m3 (DoubleRow pool)
        xn_all = xb.reshape(NT, NSUB_CH, P, C).transpose(2, 0, 1, 3)
        xn3_b = np.ascontiguousarray(xn_all[:, :NT // 2].astype(E3))
        xn4_b = np.ascontiguousarray(xn_all[:, NT // 2:].astype(E4))
        # q2 hi/lo, scaled by 256, e4m3, [p, ct, hr]
        q2s = q2[b].T * Q2SCALE                               # [C, HR]
        q2hi = q2s.astype(E4).astype(np.float32)
        q2lo = (q2s - q2hi).astype(E4)
        q2hi_b = np.ascontiguousarray(
            q2hi.astype(E4).reshape(CT, P, HR).transpose(1, 0, 2))
        q2lo_b = np.ascontiguousarray(
            q2lo.reshape(CT, P, HR).transpose(1, 0, 2))
        # mask: [p, s, r] = mask_full[r, 128s+p]
        maskt_b = np.ascontiguousarray(
            mask_full[b].T.reshape(NSUB, P, R).transpose(1, 0, 2)
            .astype(np.float16))
        in_maps.append({
            "xt": xt_b, "xn3": xn3_b, "xn4": xn4_b,
            "q2hi": q2hi_b, "q2lo": q2lo_b, "maskt": maskt_b,
        })
    return in_maps


def _get_nc():
    if "nc" not in _RUNNER_CACHE:
        _RUNNER_CACHE["nc"] = _build()
    return _RUNNER_CACHE["nc"]


def kernel(x, mask, Wq, Wk, Wv, Wp, bp, repeats=8, **_unused):
    from concourse import bass_utils

    in_maps = _prep_inputs(x, mask, Wq, Wk)
    nc = _get_nc()
    res = bass_utils.run_bass_kernel_spmd(nc, in_maps, core_ids=list(range(B)))

    # host weight epilogue in exact fp32 (O(R*C^2), no O(N) work):
    # x_cls[r, hb] = (p[:, h*8+r] / den[h*8+r]) @ Wv.T[:, hb]
    # out = x_cls @ Wp.T + bp
    wvt = np.asarray(Wv, np.float32).T
    wpt = np.asarray(Wp, np.float32).T
    bp = np.asarray(bp, np.float32)
    out = np.empty((B, R, C), np.float32)
    for b in range(B):
        den = res.results[b]["denout"].astype(np.float32).sum(axis=0)  # [HR]
        # slab [p, ct, hr] -> pool [c = ct*128+p, hr]
        poolC = (res.results[b]["slabout"].astype(np.float32)
                 .transpose(1, 0, 2).reshape(C, HR))
        x_cls = np.empty((R, C), np.float32)
        for h in range(H):
            ph = poolC[:, h * R:(h + 1) * R] / den[None, h * R:(h + 1) * R]
            x_cls[:, h * HD:(h + 1) * HD] = ph.T @ wvt[:, h * HD:(h + 1) * HD]
        out[b] = x_cls @ wpt + bp
    return out


if __name__ == "__main__":
    rng = np.random.default_rng(0)
    x = rng.standard_normal((B, N, C)).astype(np.float32)
    mask = rng.integers(0, 2, (B, R, N - R)) > 0
    s = 0.02
    Wq = (rng.standard_normal((C, C)) * s).astype(np.float32)
    Wk = (rng.standard_normal((C, C)) * s).astype(np.float32)
    Wv = (rng.standard_normal((C, C)) * s).astype(np.float32)
    Wp = (rng.standard_normal((C, C)) * s).astype(np.float32)
    bp = np.zeros(C, np.float32)
    out = kernel(x, mask, Wq, Wk, Wv, Wp, bp, 8)
    print("out", out.shape, out.dtype, np.abs(out).mean())


# revision 33
# speedup vs baseline: 1.0195x; 1.0195x over previous
"""AttentionPool kernel for 8x Trainium2 NeuronCores (Bass/Tile).

Problem (per batch b of B=8):
    q = (x[:, :8] @ Wq.T).reshape(8, 24, 64) * 64**-0.5
    k = (x @ Wk.T).reshape(4096, 24, 64)
    v = (x @ Wv.T).reshape(4096, 24, 64)
    attn = softmax(mask(q @ k.T))          # [24, 8, 4096]
    out = (attn @ v).reshape(8, 1536) @ Wp.T + bp

Sharding: data-parallel over B - one batch per NeuronCore, no collectives.

Work split: the device does ALL the O(N) token work; the host does only
O(R*C^2) weight folding (R=8):
  device  logits[n, hr] = x[n, :] . q2[hr, :]   (q2 = q*scale @ Wk folded
          on host), masked exp -> eT, unnormalized pool
          p[c, hr] = sum_n x[n, c] e[n, hr], partial denominators.
  host    q2 fold (76 MFLOP), then the weight epilogue in exact fp32:
          x_cls[r, hb] = (p[:, h*8+r]/den) @ Wv.T[:, hb], out = x_cls @ Wp.T
          + bp (0.9 GFLOP of the 312 GFLOP problem). Wv/Wp never ship to
          the device at all.

Precision/DMA budget (per core, 360 GB/s DMA model, 2.4 GHz PE):
  xt   e4m3  6.29 MB  logits GEMM in DoubleRow perf mode (2 c-tiles per
                      instruction at 0.5 cyc/row): 30.7us -> 15.4us even
                      with the q2 hi/lo residual double-pass.
  q2   e4m3 hi+lo, scaled by 256 (avoids e4m3 subnormal flush at
                      |q2|~0.02), descaled inside the Act exp (scale=1/256).
                      The lo pass cancels the q2 quantization error.
  xn   e3m4  6.29 MB  pool GEMM in normal mode, f16 moving eT (e4m3 xn
                      would cost 1.6e-2 of the 2e-2 budget vs 0.8e-2).
  out  slab [128, 12, 192] f16 + den128 [128, 192] f16 (0.64 MB).
  Host-emulated end-to-end rel err: 1.63e-2 (gate 2e-2, deterministic).

  DMA total 13.9 MB ~= 38.5us; PE = 15.4 (logits DR) + 30.7 (pool) ~= 46us
  is the critical path. All host arrays are partition-major so every
  transfer is >=512B contiguous runs. Stream order: xt0a, q2hi, q2lo,
  xt0b, mask, then xt one chunk AHEAD of xn (xt1, xn0, xt2, xn1, ...) so
  the in-order PE (logits nt, then pool nt-1) never waits on the stream.

Schedule:
  a memset-fed warmup matmul anchors the PE p-state ramp at ~0.5us so all
  real matmuls are charged at full clock (ramp model: 3us to 2.4 GHz).
  per 512-token chunk: logits per 128-token subtile (12 DoubleRow matmuls:
      6 ct-pairs x {q2hi, q2lo}) -> exp (Act, psum -> f32, scale 1/256) ->
      * mask (DVE, broadcast over heads) -> fp16 eT. Pool and den of the
      PREVIOUS chunk are emitted after this chunk's logits so the PE never
      stalls on the exp->mask chain: per c-tile psum[c128, 192] over 4
      subtiles (stationary = e3m4 x subtile, moving = eT), drained into
      one fp16 slab; den[1, 192] via DVE reduce.
  tail: den DMA fires as soon as den(7) lands; the pool-7 slab ships in
      two 6-c-tile pieces (second via the idle Act queue) so the last
      piece's issue overlaps the first piece's transfer.
"""

import numpy as np
import ml_dtypes

B, N, C = 8, 4096, 1536
H, HD, R = 24, 64, 8
HR = H * R           # 192 (h, r) pairs, index hr = h*R + r
SCALE = HD ** -0.5
P = 128
CT = C // P          # 12 contraction tiles
CTP = CT // 2        # 6 DoubleRow ct-pairs
NCHUNK = 512
NSUB_CH = NCHUNK // P  # 4 subtiles per chunk
NT = N // NCHUNK     # 8 chunks
NSUB = N // P        # 32 token subtiles total

Q2SCALE = 256.0      # q2 pre-scale (e4m3 subnormal avoidance)

_RUNNER_CACHE = {}


def _build():
    import concourse.mybir as mybir
    import concourse.tile as tile
    from concourse import bacc

    F32 = mybir.dt.float32
    F16 = mybir.dt.float16
    F8E3 = mybir.dt.float8e3
    F8E4 = mybir.dt.float8e4
    MULT = mybir.AluOpType.mult
    SUB = mybir.AluOpType.subtract
    ADD = mybir.AluOpType.add
    EXP = mybir.ActivationFunctionType.Exp
    DR = mybir.MatmulPerfMode.DoubleRow

    nc = bacc.Bacc(None, target_bir_lowering=False)
    # x.T partition-major: [p, nt, half, ct, 256] = x[512nt+256h+n', 128ct+p]
    xt = nc.dram_tensor("xt", [P, NT, 2, CT, NCHUNK // 2], F8E4,
                        kind="ExternalInput")
    # x partition-major: [p, nt, s, c] = x[512nt+128s+p, c]; chunks 0-3
    # ship e3m4 (pool in normal mode), chunks 4-7 e4m3 (pool in DoubleRow
    # with hi/lo e4m3 eT)
    xn3 = nc.dram_tensor("xn3", [P, NT // 2, NSUB_CH, C], F8E3,
                         kind="ExternalInput")
    xn4 = nc.dram_tensor("xn4", [P, NT // 2, NSUB_CH, C], F8E4,
                         kind="ExternalInput")
    q2hi = nc.dram_tensor("q2hi", [P, CT, HR], F8E4, kind="ExternalInput")
    q2lo = nc.dram_tensor("q2lo", [P, CT, HR], F8E4, kind="ExternalInput")
    # mask partition-major: [p, s, r] = mask_full[r, 128s+p]
    maskt = nc.dram_tensor("maskt", [P, NSUB, R], F16, kind="ExternalInput")
    slabout = nc.dram_tensor("slabout", [P, CT, HR], F16,
                             kind="ExternalOutput")
    denout = nc.dram_tensor("denout", [P, HR], F16, kind="ExternalOutput")

    with tile.TileContext(nc) as tc:
        with (
            tc.tile_pool(name="pper", bufs=1) as pper,      # persistent
            tc.tile_pool(name="pxt", bufs=4) as pxt,        # xT chunks
            tc.tile_pool(name="pxn", bufs=4) as pxn,
            tc.tile_pool(name="pxn4", bufs=4) as pxn4,        # x chunks
            tc.tile_pool(name="pexp", bufs=2) as pexp,
            tc.tile_pool(name="ptmp", bufs=3) as ptmp,
            tc.tile_pool(name="pslab", bufs=1) as pslab,
            tc.tile_pool(name="ps_l", bufs=2, space="PSUM") as ps_l,
            tc.tile_pool(name="ps_p", bufs=4, space="PSUM") as ps_p,
            tc.tile_pool(name="ps_d", bufs=1, space="PSUM") as ps_d,
        ):
            # ---------- persistent tiles ----------
            q2hi_sb = pper.tile([P, CT, HR], F8E4, tag="q2hi")
            q2lo_sb = pper.tile([P, CT, HR], F8E4, tag="q2lo")
            maskt_sb = pper.tile([P, NSUB, R], F16, tag="maskt")
            eT = pper.tile([P, NSUB, HR], F16, tag="eT")        # masked exp
            # e4m3 hi/lo split of eT for the DoubleRow pool chunks (si>=16)
            eH = pper.tile([P, NSUB // 2, HR], F8E4, tag="eH")
            eL = pper.tile([P, NSUB // 2, HR], F8E4, tag="eL")
            den128 = pper.tile([P, HR], F16, tag="den128")
            warm = pper.tile([P, 8], F16, tag="warm")

            # ---------- DMA emission helpers (order == queue order) -------
            # xt chunk tile: [p, half, ct, 256]; subtile s of the chunk is
            # [:, s // 2, :, (s % 2) * P:(s % 2 + 1) * P]
            xt_ch0 = pxt.tile([P, 2, CT, NCHUNK // 2], F8E4, tag="xt")

            def _xt_chunk(xt_ch, nt):
                nc.sync.dma_start(xt_ch, xt[:, nt])

            xn_ch0 = pxn.tile([P, NSUB_CH, C], F8E3, tag="xn")

            def _xn_chunk(xn_ch, nt):
                if nt < NT // 2:
                    nc.sync.dma_start(xn_ch, xn3[:, nt])
                else:
                    nc.sync.dma_start(xn_ch, xn4[:, nt - NT // 2])

            # warmup: memset-fed tiny matmuls anchor the PE p-state ramp at
            # ~0.5us, long before the first data-gated matmul, so all real
            # matmuls run at full clock (the ramp model needs 3us of busy
            # history to reach 2.4 GHz)
            nc.vector.memset(warm, 1.0)
            for w in range(3):
                ps_w = ps_d.tile([P, 512], F32, tag="pd")
                nc.tensor.matmul(
                    ps_w[0:1, 0:1], warm[:, w:w + 1], warm[:, w:w + 1],
                    start=True, stop=True)

            # startup stream, ordered by need-time
            nc.sync.dma_start(xt_ch0[:, 0], xt[:, 0, 0])
            nc.sync.dma_start(q2hi_sb, q2hi[:])
            nc.sync.dma_start(q2lo_sb, q2lo[:])
            nc.sync.dma_start(xt_ch0[:, 1], xt[:, 0, 1])
            nc.sync.dma_start(maskt_sb, maskt[:])

            # ---------- per-chunk pipeline ----------
            def emit_logits(nt, xt_ch):
                for s in range(NSUB_CH):
                    si = nt * NSUB_CH + s
                    ps = ps_l.tile([P, 512], F32, tag="pl")
                    lT = ps[:, 0:HR]
                    xsub = xt_ch[:, s // 2, :, (s % 2) * P:(s % 2 + 1) * P]
                    for qi, q2sb in enumerate((q2hi_sb, q2lo_sb)):
                        for t in range(CTP):
                            nc.tensor.matmul(
                                lT,
                                xsub[:, 2 * t:2 * t + 2],
                                q2sb[:, 2 * t:2 * t + 2],
                                start=(qi == 0 and t == 0),
                                stop=(qi == 1 and t == CTP - 1),
                                perf_mode=DR)
                    # exp descales the q2 pre-scale: e = exp(psum / 256)
                    nc.scalar.activation(eT[:, si], lT, EXP, scale=1.0 / Q2SCALE)
                    # in-place 0/1 mask: all operands fp16+SBUF, so the DVE
                    # runs in 2x mode; product is exact (mask is 0 or 1)
                    nc.vector.tensor_tensor(
                        eT[:, si].rearrange("p (h r) -> p h r", h=H),
                        eT[:, si].rearrange("p (h r) -> p h r", h=H),
                        maskt_sb[:, si, None, :].to_broadcast((P, H, R)),
                        MULT)
                    if si >= NSUB // 2:
                        # hi/lo e4m3 split for the DoubleRow pool: hi on the
                        # Act engine, residual lo = eT - hi on the DVE
                        sj = si - NSUB // 2
                        nc.scalar.copy(eH[:, sj], eT[:, si])
                        nc.vector.tensor_tensor(
                            eL[:, sj], eT[:, si], eH[:, sj], SUB)

            def emit_pool(nt, xn_ch, slab, ct_lo=0, ct_hi=CT):
                # pool psum per c-tile; slab accumulates chunks in fp16
                # (adds cost ~5e-4 relative - fine). First chunk drains as
                # copies split across DVE and Act; later chunks add on DVE.
                dr = nt >= NT // 2
                for ct in range(ct_lo, ct_hi):
                    ps = ps_p.tile([P, 512], F32, tag="pp")
                    pch = ps[:, 0:HR]
                    if dr:
                        # DoubleRow: k-tile pairs are subtile pairs; two
                        # passes (eH, eL) cancel the e4m3 eT quantization
                        sj0 = (nt - NT // 2) * NSUB_CH
                        for ei, esb in enumerate((eH, eL)):
                            for u in range(NSUB_CH // 2):
                                nc.tensor.matmul(
                                    pch,
                                    xn_ch[:, 2 * u:2 * u + 2,
                                          ct * P:(ct + 1) * P],
                                    esb[:, sj0 + 2 * u:sj0 + 2 * u + 2],
                                    start=(ei == 0 and u == 0),
                                    stop=(ei == 1 and u == NSUB_CH // 2 - 1),
                                    perf_mode=DR)
                    else:
                        for s in range(NSUB_CH):
                            si = nt * NSUB_CH + s
                            nc.tensor.matmul(
                                pch,
                                xn_ch[:, s, ct * P:(ct + 1) * P],
                                eT[:, si],
                                start=(s == 0), stop=(s == NSUB_CH - 1))
                    # drains split across engines so the DVE isn't the
                    # pacer: even cts add on DVE; odd cts route PSUM->SBUF
                    # through an Act copy (GPSIMD can't read PSUM) and add
                    # SBUF->SBUF on the idle GPSIMD
                    if nt == 0:
                        if ct % 2 == 0:
                            nc.vector.tensor_copy(slab[:, ct], pch)
                        else:
                            nc.scalar.copy(slab[:, ct], pch)
                    elif ct % 2 == 0:
                        nc.vector.tensor_add(slab[:, ct], slab[:, ct], pch)
                    else:
                        tmp = ptmp.tile([P, HR], F16, tag="ptmp",
                                        name=f"tmp_{nt}_{ct}")
                        nc.scalar.copy(tmp, pch)
                        nc.gpsimd.tensor_add(slab[:, ct], slab[:, ct], tmp)

            def emit_den(nt):
                # per-partition partial denominators on the DVE (idle
                # capacity): innermost-axis reduce on a strided view; the
                # cross-partition sum happens on the host in fp32.
                sl = eT[:, nt * NSUB_CH:(nt + 1) * NSUB_CH].rearrange(
                    "p s h -> p h s")
                with nc.allow_low_precision(reason="den rel err ~5e-4"):
                    if nt == 0:
                        nc.vector.tensor_reduce(
                            den128, sl, mybir.AxisListType.X, ADD)
                    else:
                        dpart = pexp.tile([P, HR], F16, tag="dpart")
                        nc.vector.tensor_reduce(
                            dpart, sl, mybir.AxisListType.X, ADD)
                        nc.vector.tensor_add(den128, den128, dpart)

            slab = pslab.tile([P, CT, HR], F16, tag="slab", name="slab")
            xns = []
            xt_ch = xt_ch0
            for nt in range(NT):
                # stream xt one chunk AHEAD of xn: the PE runs logits(nt)
                # then pool(nt-1), so it needs xt(nt) before xn(nt-1)
                if nt + 1 < NT:
                    xt_nx = pxt.tile([P, 2, CT, NCHUNK // 2], F8E4, tag="xt")
                    _xt_chunk(xt_nx, nt + 1)
                else:
                    xt_nx = None
                if nt >= NT // 2:
                    xn_ch = pxn4.tile([P, NSUB_CH, C], F8E4, tag="xn4",
                                      name=f"xn_ch{nt}")
                    _xn_chunk(xn_ch, nt)
                elif nt > 0:
                    xn_ch = pxn.tile([P, NSUB_CH, C], F8E3, tag="xn",
                                     name=f"xn_ch{nt}")
                    _xn_chunk(xn_ch, nt)
                else:
                    # chunk 0's xn loads in halves so pool-0 (the head of
                    # the PE's steady pipeline) starts a hop earlier
                    xn_ch = xn_ch0
                    nc.sync.dma_start(xn_ch0[:, 0:2], xn3[:, 0, 0:2])
                    nc.sync.dma_start(xn_ch0[:, 2:4], xn3[:, 0, 2:4])
                emit_logits(nt, xt_ch)
                if nt > 0:
                    emit_pool(nt - 1, xns[nt - 1], slab)
                    emit_den(nt - 1)
                xns.append(xn_ch)
                xt_ch = xt_nx

            # ---------- tail ----------
            # den(7) completes on the DVE as soon as eT(7) is masked; its
            # DMA fires while the pool-7 matmuls still run. The slab ships
            # in two 6-c-tile pieces; the second goes through the idle Act
            # queue so both issue chains overlap.
            emit_den(NT - 1)
            nc.sync.dma_start(denout[:], den128)
            emit_pool(NT - 1, xns[NT - 1], slab, 0, 6)
            nc.sync.dma_start(slabout[:, 0:6], slab[:, 0:6])
            emit_pool(NT - 1, xns[NT - 1], slab, 6, CT)
            nc.sync.dma_start(slabout[:, 6:CT], slab[:, 6:CT])

    nc.compile()
    return nc


def _prep_inputs(x, mask, Wq, Wk):
    """Host-side sharding + layout prep. Returns per-core in_maps.

    The 8-token q projection and its fold through Wk (q2 = q*scale @
    Wk[head rows]) happen here: 76 MFLOP of the 312 GFLOP problem."""
    E3 = ml_dtypes.float8_e3m4
    E4 = ml_dtypes.float8_e4m3

    x = np.asarray(x, dtype=np.float32)
    Wq = np.asarray(Wq, np.float32)
    Wk = np.asarray(Wk, np.float32)

    mask = np.asarray(mask)
    mask_full = np.empty((B, R, N), np.float32)
    mask_full[:, :, :R] = np.eye(R, dtype=np.float32)[None]
    mask_full[:, :, R:] = mask.astype(np.float32)

    # q2[b, hr, c] = sum_d q[b, r, h, d]*SCALE * Wk[h*HD+d, c]
    q = np.einsum('brc,dc->brd', x[:, :R], Wq) * SCALE        # [B, R, C]
    q2 = np.einsum('brhd,hdc->bhrc',
                   q.reshape(B, R, H, HD), Wk.reshape(H, HD, C))
    q2 = q2.reshape(B, HR, C)

    in_maps = []
    for b in range(B):
        xb = np.clip(x[b], -15.0, 15.0)
        # xt: [p, nt, half, ct, 256] = x[512nt+256h+n', 128ct+p], e4m3
        xt_b = np.ascontiguousarray(
            xb.T.reshape(CT, P, NT, 2, NCHUNK // 2)
            .transpose(1, 2, 3, 0, 4).astype(E4))
        # xn: [p, nt, s, c] = x[512nt+128s+p, c]; chunks 0-3 e3m4
        # (normal-mode pool), chunks 4-7 e4m3 (DoubleRow pool)
        xn_all = xb.reshape(NT, NSUB_CH, P, C).transpose(2, 0, 1, 3)
        xn3_b = np.ascontiguousarray(xn_all[:, :NT // 2].astype(E3))
        xn4_b = np.ascontiguousarray(xn_all[:, NT // 2:].astype(E4))
        # q2 hi/lo, scaled by 256, e4m3, [p, ct, hr]
        q2s = q2[b].T * Q2SCALE                               # [C, HR]
        q2hi = q2s.astype(E4).astype(np.float32)
        q2lo = (q2s - q2hi).astype(E4)
        q2hi_b = np.ascontiguousarray(
            q2hi.astype(E4).reshape(CT, P, HR).transpose(1, 0, 2))
        q2lo_b = np.ascontiguousarray(
            q2lo.reshape(CT, P, HR).transpose(1, 0, 2))
        # mask: [p, s, r] = mask_full[r, 128s+p]
        maskt_b = np.ascontiguousarray(
            mask_full[b].T.reshape(NSUB, P, R).transpose(1, 0, 2)
            .astype(np.float16))
        in_maps.append({
            "xt": xt_b, "xn3": xn3_b, "xn4": xn4_b,
            "q2hi": q2hi_b, "q2lo": q2lo_b, "maskt": maskt_b,
        })
    return in_maps


def _get_nc():
    if "nc" not in _RUNNER_CACHE:
        _RUNNER_CACHE["nc"] = _build()
    return _RUNNER_CACHE["nc"]


def kernel(x, mask, Wq, Wk, Wv, Wp, bp, repeats=8, **_unused):
    from concourse import bass_utils

    in_maps = _prep_inputs(x, mask, Wq, Wk)
    nc = _get_nc()
    res = bass_utils.run_bass_kernel_spmd(nc, in_maps, core_ids=list(range(B)))

    # host weight epilogue in exact fp32 (O(R*C^2), no O(N) work):
    # x_cls[r, hb] = (p[:, h*8+r] / den[h*8+r]) @ Wv.T[:, hb]
    # out = x_cls @ Wp.T + bp
    wvt = np.asarray(Wv, np.float32).T
    wpt = np.asarray(Wp, np.float32).T
    bp = np.asarray(bp, np.float32)
    out = np.empty((B, R, C), np.float32)
    for b in range(B):
        den = res.results[b]["denout"].astype(np.float32).sum(axis=0)  # [HR]
        # slab [p, ct, hr] -> pool [c = ct*128+p, hr]
        poolC = (res.results[b]["slabout"].astype(np.float32)
                 .transpose(1, 0, 2).reshape(C, HR))
        x_cls = np.empty((R, C), np.float32)
        for h in range(H):
            ph = poolC[:, h * R:(h + 1) * R] / den[None, h * R:(h + 1) * R]
            x_cls[:, h * HD:(h + 1) * HD] = ph.T @ wvt[:, h * HD:(h + 1) * HD]
        out[b] = x_cls @ wpt + bp
    return out


if __name__ == "__main__":
    rng = np.random.default_rng(0)
    x = rng.standard_normal((B, N, C)).astype(np.float32)
    mask = rng.integers(0, 2, (B, R, N - R)) > 0
    s = 0.02
    Wq = (rng.standard_normal((C, C)) * s).astype(np.float32)
    Wk = (rng.standard_normal((C, C)) * s).astype(np.float32)
    Wv = (rng.standard_normal((C, C)) * s).astype(np.float32)
    Wp = (rng.standard_normal((C, C)) * s).astype(np.float32)
    bp = np.zeros(C, np.float32)
    out = kernel(x, mask, Wq, Wk, Wv, Wp, bp, 8)
    print("out", out.shape, out.dtype, np.abs(out).mean())


# revision 34
# speedup vs baseline: 1.0450x; 1.0251x over previous
"""AttentionPool kernel for 8x Trainium2 NeuronCores (Bass/Tile).

Problem (per batch b of B=8):
    q = (x[:, :8] @ Wq.T).reshape(8, 24, 64) * 64**-0.5
    k = (x @ Wk.T).reshape(4096, 24, 64)
    v = (x @ Wv.T).reshape(4096, 24, 64)
    attn = softmax(mask(q @ k.T))          # [24, 8, 4096]
    out = (attn @ v).reshape(8, 1536) @ Wp.T + bp

Sharding: data-parallel over B - one batch per NeuronCore, no collectives.

Work split: the device does ALL the O(N) token work; the host does only
O(R*C^2) weight folding (R=8):
  device  logits[n, hr] = x[n, :] . q2[hr, :]   (q2 = q*scale @ Wk folded
          on host), masked exp -> eT, unnormalized pool
          p[c, hr] = sum_n x[n, c] e[n, hr], partial denominators.
  host    q2 fold (76 MFLOP), then the weight epilogue in exact fp32:
          x_cls[r, hb] = (p[:, h*8+r]/den) @ Wv.T[:, hb], out = x_cls @ Wp.T
          + bp (0.9 GFLOP of the 312 GFLOP problem). Wv/Wp never ship to
          the device at all.

Precision/DMA budget (per core, 360 GB/s DMA model, 2.4 GHz PE):
  xt   e4m3  6.29 MB  logits GEMM in DoubleRow perf mode (2 c-tiles per
                      instruction at 0.5 cyc/row): 30.7us -> 15.4us even
                      with the q2 hi/lo residual double-pass.
  q2   e4m3 hi+lo, scaled by 256 (avoids e4m3 subnormal flush at
                      |q2|~0.02), descaled inside the Act exp (scale=1/256).
                      The lo pass cancels the q2 quantization error.
  xn   e3m4  6.29 MB  pool GEMM in normal mode, f16 moving eT (e4m3 xn
                      would cost 1.6e-2 of the 2e-2 budget vs 0.8e-2).
  out  slab [128, 12, 192] f16 + den128 [128, 192] f16 (0.64 MB).
  Host-emulated end-to-end rel err: 1.63e-2 (gate 2e-2, deterministic).

  DMA total 13.9 MB ~= 38.5us; PE = 15.4 (logits DR) + 30.7 (pool) ~= 46us
  is the critical path. All host arrays are partition-major so every
  transfer is >=512B contiguous runs. Stream order: xt0a, q2hi, q2lo,
  xt0b, mask, then xt one chunk AHEAD of xn (xt1, xn0, xt2, xn1, ...) so
  the in-order PE (logits nt, then pool nt-1) never waits on the stream.

Schedule:
  a memset-fed warmup matmul anchors the PE p-state ramp at ~0.5us so all
  real matmuls are charged at full clock (ramp model: 3us to 2.4 GHz).
  per 512-token chunk: logits per 128-token subtile (12 DoubleRow matmuls:
      6 ct-pairs x {q2hi, q2lo}) -> exp (Act, psum -> f32, scale 1/256) ->
      * mask (DVE, broadcast over heads) -> fp16 eT. Pool and den of the
      PREVIOUS chunk are emitted after this chunk's logits so the PE never
      stalls on the exp->mask chain: per c-tile psum[c128, 192] over 4
      subtiles (stationary = e3m4 x subtile, moving = eT), drained into
      one fp16 slab; den[1, 192] via DVE reduce.
  tail: den DMA fires as soon as den(7) lands; the pool-7 slab ships in
      two 6-c-tile pieces (second via the idle Act queue) so the last
      piece's issue overlaps the first piece's transfer.
"""

import numpy as np
import ml_dtypes

B, N, C = 8, 4096, 1536
H, HD, R = 24, 64, 8
HR = H * R           # 192 (h, r) pairs, index hr = h*R + r
SCALE = HD ** -0.5
P = 128
CT = C // P          # 12 contraction tiles
CTP = CT // 2        # 6 DoubleRow ct-pairs
NCHUNK = 512
NSUB_CH = NCHUNK // P  # 4 subtiles per chunk
NT = N // NCHUNK     # 8 chunks
NSUB = N // P        # 32 token subtiles total

Q2SCALE = 256.0      # q2 pre-scale (e4m3 subnormal avoidance)

_RUNNER_CACHE = {}


def _build():
    import concourse.mybir as mybir
    import concourse.tile as tile
    from concourse import bacc

    F32 = mybir.dt.float32
    F16 = mybir.dt.float16
    F8E3 = mybir.dt.float8e3
    F8E4 = mybir.dt.float8e4
    MULT = mybir.AluOpType.mult
    SUB = mybir.AluOpType.subtract
    ADD = mybir.AluOpType.add
    EXP = mybir.ActivationFunctionType.Exp
    DR = mybir.MatmulPerfMode.DoubleRow

    nc = bacc.Bacc(None, target_bir_lowering=False)
    # x.T partition-major: [p, nt, half, ct, 256] = x[512nt+256h+n', 128ct+p]
    xt = nc.dram_tensor("xt", [P, NT, 2, CT, NCHUNK // 2], F8E4,
                        kind="ExternalInput")
    # x partition-major: [p, nt, s, c] = x[512nt+128s+p, c]; chunks 0-3
    # ship e3m4 (pool in normal mode), chunks 4-7 e4m3 (pool in DoubleRow
    # with hi/lo e4m3 eT)
    xn3 = nc.dram_tensor("xn3", [P, NT // 2, NSUB_CH, C], F8E3,
                         kind="ExternalInput")
    xn4 = nc.dram_tensor("xn4", [P, NT // 2, NSUB_CH, C], F8E4,
                         kind="ExternalInput")
    q2hi = nc.dram_tensor("q2hi", [P, CT, HR], F8E4, kind="ExternalInput")
    q2lo = nc.dram_tensor("q2lo", [P, CT, HR], F8E4, kind="ExternalInput")
    # mask partition-major: [p, s, r] = mask_full[r, 128s+p]
    maskt = nc.dram_tensor("maskt", [P, NSUB, R], F16, kind="ExternalInput")
    slabout = nc.dram_tensor("slabout", [P, CT, HR], F16,
                             kind="ExternalOutput")
    denout = nc.dram_tensor("denout", [P, HR], F16, kind="ExternalOutput")

    with tile.TileContext(nc) as tc:
        with (
            tc.tile_pool(name="pper", bufs=1) as pper,      # persistent
            tc.tile_pool(name="pxt", bufs=4) as pxt,        # xT chunks
            tc.tile_pool(name="pxn", bufs=4) as pxn,
            tc.tile_pool(name="pxn4", bufs=4) as pxn4,        # x chunks
            tc.tile_pool(name="pexp", bufs=2) as pexp,
            tc.tile_pool(name="ptmp", bufs=3) as ptmp,
            tc.tile_pool(name="pslab", bufs=1) as pslab,
            tc.tile_pool(name="ps_l", bufs=2, space="PSUM") as ps_l,
            tc.tile_pool(name="ps_p", bufs=5, space="PSUM") as ps_p,
            tc.tile_pool(name="ps_d", bufs=1, space="PSUM") as ps_d,
        ):
            # ---------- persistent tiles ----------
            q2hi_sb = pper.tile([P, CT, HR], F8E4, tag="q2hi")
            q2lo_sb = pper.tile([P, CT, HR], F8E4, tag="q2lo")
            maskt_sb = pper.tile([P, NSUB, R], F16, tag="maskt")
            eT = pper.tile([P, NSUB, HR], F16, tag="eT")        # masked exp
            # e4m3 hi/lo split of eT for the DoubleRow pool chunks (si>=16)
            eH = pper.tile([P, NSUB // 2, HR], F8E4, tag="eH")
            eL = pper.tile([P, NSUB // 2, HR], F8E4, tag="eL")
            den128 = pper.tile([P, HR], F16, tag="den128")
            warm = pper.tile([P, 8], F16, tag="warm")

            # ---------- DMA emission helpers (order == queue order) -------
            # xt chunk tile: [p, half, ct, 256]; subtile s of the chunk is
            # [:, s // 2, :, (s % 2) * P:(s % 2 + 1) * P]
            xt_ch0 = pxt.tile([P, 2, CT, NCHUNK // 2], F8E4, tag="xt")

            def _xt_chunk(xt_ch, nt):
                nc.sync.dma_start(xt_ch, xt[:, nt])

            xn_ch0 = pxn.tile([P, NSUB_CH, C], F8E3, tag="xn")

            def _xn_chunk(xn_ch, nt):
                if nt < NT // 2:
                    nc.sync.dma_start(xn_ch, xn3[:, nt])
                else:
                    nc.sync.dma_start(xn_ch, xn4[:, nt - NT // 2])

            # warmup: memset-fed tiny matmuls anchor the PE p-state ramp at
            # ~0.5us, long before the first data-gated matmul, so all real
            # matmuls run at full clock (the ramp model needs 3us of busy
            # history to reach 2.4 GHz)
            nc.vector.memset(warm, 1.0)
            for w in range(3):
                ps_w = ps_d.tile([P, 512], F32, tag="pd")
                nc.tensor.matmul(
                    ps_w[0:1, 0:1], warm[:, w:w + 1], warm[:, w:w + 1],
                    start=True, stop=True)

            # startup stream, ordered by need-time
            nc.sync.dma_start(xt_ch0[:, 0], xt[:, 0, 0])
            nc.sync.dma_start(q2hi_sb, q2hi[:])
            nc.sync.dma_start(q2lo_sb, q2lo[:])
            nc.sync.dma_start(xt_ch0[:, 1], xt[:, 0, 1])
            nc.sync.dma_start(maskt_sb, maskt[:])

            # ---------- per-chunk pipeline ----------
            def emit_logits(nt, xt_ch):
                for s in range(NSUB_CH):
                    si = nt * NSUB_CH + s
                    ps = ps_l.tile([P, 512], F32, tag="pl")
                    lT = ps[:, 0:HR]
                    xsub = xt_ch[:, s // 2, :, (s % 2) * P:(s % 2 + 1) * P]
                    for qi, q2sb in enumerate((q2hi_sb, q2lo_sb)):
                        for t in range(CTP):
                            nc.tensor.matmul(
                                lT,
                                xsub[:, 2 * t:2 * t + 2],
                                q2sb[:, 2 * t:2 * t + 2],
                                start=(qi == 0 and t == 0),
                                stop=(qi == 1 and t == CTP - 1),
                                perf_mode=DR)
                    # exp descales the q2 pre-scale: e = exp(psum / 256)
                    nc.scalar.activation(eT[:, si], lT, EXP, scale=1.0 / Q2SCALE)
                    # in-place 0/1 mask: all operands fp16+SBUF, so the DVE
                    # runs in 2x mode; product is exact (mask is 0 or 1)
                    nc.vector.tensor_tensor(
                        eT[:, si].rearrange("p (h r) -> p h r", h=H),
                        eT[:, si].rearrange("p (h r) -> p h r", h=H),
                        maskt_sb[:, si, None, :].to_broadcast((P, H, R)),
                        MULT)
                    if si >= NSUB // 2:
                        # hi/lo e4m3 split for the DoubleRow pool: hi on the
                        # Act engine, residual lo = eT - hi on the DVE
                        sj = si - NSUB // 2
                        nc.scalar.copy(eH[:, sj], eT[:, si])
                        nc.vector.tensor_tensor(
                            eL[:, sj], eT[:, si], eH[:, sj], SUB)

            def emit_pool(nt, xn_ch, slab, ct_lo=0, ct_hi=CT):
                # pool psum per c-tile; slab accumulates chunks in fp16
                # (adds cost ~5e-4 relative - fine). First chunk drains as
                # copies split across DVE and Act; later chunks add on DVE.
                dr = nt >= NT // 2
                for ct in range(ct_lo, ct_hi):
                    ps = ps_p.tile([P, 512], F32, tag="pp")
                    pch = ps[:, 0:HR]
                    if dr:
                        # DoubleRow: k-tile pairs are subtile pairs; two
                        # passes (eH, eL) cancel the e4m3 eT quantization
                        sj0 = (nt - NT // 2) * NSUB_CH
                        for ei, esb in enumerate((eH, eL)):
                            for u in range(NSUB_CH // 2):
                                nc.tensor.matmul(
                                    pch,
                                    xn_ch[:, 2 * u:2 * u + 2,
                                          ct * P:(ct + 1) * P],
                                    esb[:, sj0 + 2 * u:sj0 + 2 * u + 2],
                                    start=(ei == 0 and u == 0),
                                    stop=(ei == 1 and u == NSUB_CH // 2 - 1),
                                    perf_mode=DR)
                    else:
                        for s in range(NSUB_CH):
                            si = nt * NSUB_CH + s
                            nc.tensor.matmul(
                                pch,
                                xn_ch[:, s, ct * P:(ct + 1) * P],
                                eT[:, si],
                                start=(s == 0), stop=(s == NSUB_CH - 1))
                    # drains split across engines so the DVE isn't the
                    # pacer: even cts add on DVE; odd cts route PSUM->SBUF
                    # through an Act copy (GPSIMD can't read PSUM) and add
                    # SBUF->SBUF on the idle GPSIMD
                    if nt == 0:
                        if ct % 2 == 0:
                            nc.vector.tensor_copy(slab[:, ct], pch)
                        else:
                            nc.scalar.copy(slab[:, ct], pch)
                    elif ct % 2 == 0:
                        nc.vector.tensor_add(slab[:, ct], slab[:, ct], pch)
                    else:
                        tmp = ptmp.tile([P, HR], F16, tag="ptmp",
                                        name=f"tmp_{nt}_{ct}")
                        nc.scalar.copy(tmp, pch)
                        nc.gpsimd.tensor_add(slab[:, ct], slab[:, ct], tmp)

            def emit_den(nt):
                # per-partition partial denominators on the DVE (idle
                # capacity): innermost-axis reduce on a strided view; the
                # cross-partition sum happens on the host in fp32.
                sl = eT[:, nt * NSUB_CH:(nt + 1) * NSUB_CH].rearrange(
                    "p s h -> p h s")
                with nc.allow_low_precision(reason="den rel err ~5e-4"):
                    if nt == 0:
                        nc.vector.tensor_reduce(
                            den128, sl, mybir.AxisListType.X, ADD)
                    else:
                        dpart = pexp.tile([P, HR], F16, tag="dpart")
                        nc.vector.tensor_reduce(
                            dpart, sl, mybir.AxisListType.X, ADD)
                        nc.vector.tensor_add(den128, den128, dpart)

            slab = pslab.tile([P, CT, HR], F16, tag="slab", name="slab")
            xns = []
            xt_ch = xt_ch0
            for nt in range(NT):
                # stream xt one chunk AHEAD of xn: the PE runs logits(nt)
                # then pool(nt-1), so it needs xt(nt) before xn(nt-1)
                if nt + 1 < NT:
                    xt_nx = pxt.tile([P, 2, CT, NCHUNK // 2], F8E4, tag="xt")
                    _xt_chunk(xt_nx, nt + 1)
                else:
                    xt_nx = None
                if nt >= NT // 2:
                    xn_ch = pxn4.tile([P, NSUB_CH, C], F8E4, tag="xn4",
                                      name=f"xn_ch{nt}")
                    _xn_chunk(xn_ch, nt)
                elif nt > 0:
                    xn_ch = pxn.tile([P, NSUB_CH, C], F8E3, tag="xn",
                                     name=f"xn_ch{nt}")
                    _xn_chunk(xn_ch, nt)
                else:
                    # chunk 0's xn loads in halves so pool-0 (the head of
                    # the PE's steady pipeline) starts a hop earlier
                    xn_ch = xn_ch0
                    nc.sync.dma_start(xn_ch0[:, 0:2], xn3[:, 0, 0:2])
                    nc.sync.dma_start(xn_ch0[:, 2:4], xn3[:, 0, 2:4])
                emit_logits(nt, xt_ch)
                if nt > 0:
                    emit_pool(nt - 1, xns[nt - 1], slab)
                    emit_den(nt - 1)
                xns.append(xn_ch)
                xt_ch = xt_nx

            # ---------- tail ----------
            # den(7) completes on the DVE as soon as eT(7) is masked; its
            # DMA fires while the pool-7 matmuls still run. The slab ships
            # in two 6-c-tile pieces; the second goes through the idle Act
            # queue so both issue chains overlap.
            emit_den(NT - 1)
            nc.sync.dma_start(denout[:], den128)
            emit_pool(NT - 1, xns[NT - 1], slab, 0, 6)
            nc.sync.dma_start(slabout[:, 0:6], slab[:, 0:6])
            emit_pool(NT - 1, xns[NT - 1], slab, 6, CT)
            nc.sync.dma_start(slabout[:, 6:CT], slab[:, 6:CT])

    nc.compile()
    return nc


def _prep_inputs(x, mask, Wq, Wk):
    """Host-side sharding + layout prep. Returns per-core in_maps.

    The 8-token q projection and its fold through Wk (q2 = q*scale @
    Wk[head rows]) happen here: 76 MFLOP of the 312 GFLOP problem."""
    E3 = ml_dtypes.float8_e3m4
    E4 = ml_dtypes.float8_e4m3

    x = np.asarray(x, dtype=np.float32)
    Wq = np.asarray(Wq, np.float32)
    Wk = np.asarray(Wk, np.float32)

    mask = np.asarray(mask)
    mask_full = np.empty((B, R, N), np.float32)
    mask_full[:, :, :R] = np.eye(R, dtype=np.float32)[None]
    mask_full[:, :, R:] = mask.astype(np.float32)

    # q2[b, hr, c] = sum_d q[b, r, h, d]*SCALE * Wk[h*HD+d, c]
    q = np.einsum('brc,dc->brd', x[:, :R], Wq) * SCALE        # [B, R, C]
    q2 = np.einsum('brhd,hdc->bhrc',
                   q.reshape(B, R, H, HD), Wk.reshape(H, HD, C))
    q2 = q2.reshape(B, HR, C)

    in_maps = []
    for b in range(B):
        xb = np.clip(x[b], -15.0, 15.0)
        # xt: [p, nt, half, ct, 256] = x[512nt+256h+n', 128ct+p], e4m3
        xt_b = np.ascontiguousarray(
            xb.T.reshape(CT, P, NT, 2, NCHUNK // 2)
            .transpose(1, 2, 3, 0, 4).astype(E4))
        # xn: [p, nt, s, c] = x[512nt+128s+p, c]; chunks 0-3 e3m4
        # (normal-mode pool), chunks 4-7 e4m3 (DoubleRow pool)
        xn_all = xb.reshape(NT, NSUB_CH, P, C).transpose(2, 0, 1, 3)
        xn3_b = np.ascontiguousarray(xn_all[:, :NT // 2].astype(E3))
        xn4_b = np.ascontiguousarray(xn_all[:, NT // 2:].astype(E4))
        # q2 hi/lo, scaled by 256, e4m3, [p, ct, hr]
        q2s = q2[b].T * Q2SCALE                               # [C, HR]
        q2hi = q2s.astype(E4).astype(np.float32)
        q2lo = (q2s - q2hi).astype(E4)
        q2hi_b = np.ascontiguousarray(
            q2hi.astype(E4).reshape(CT, P, HR).transpose(1, 0, 2))
        q2lo_b = np.ascontiguousarray(
            q2lo.reshape(CT, P, HR).transpose(1, 0, 2))
        # mask: [p, s, r] = mask_full[r, 128s+p]
        maskt_b = np.ascontiguousarray(
            mask_full[b].T.reshape(NSUB, P, R).transpose(1, 0, 2)
            .astype(np.float16))
        in_maps.append({
            "xt": xt_b, "xn3": xn3_b, "xn4": xn4_b,
            "q2hi": q2hi_b, "q2lo": q2lo_b, "maskt": maskt_b,
        })
    return in_maps


def _get_nc():
    if "nc" not in _RUNNER_CACHE:
        _RUNNER_CACHE["nc"] = _build()
    return _RUNNER_CACHE["nc"]


def kernel(x, mask, Wq, Wk, Wv, Wp, bp, repeats=8, **_unused):
    from concourse import bass_utils

    in_maps = _prep_inputs(x, mask, Wq, Wk)
    nc = _get_nc()
    res = bass_utils.run_bass_kernel_spmd(nc, in_maps, core_ids=list(range(B)))

    # host weight epilogue in exact fp32 (O(R*C^2), no O(N) work):
    # x_cls[r, hb] = (p[:, h*8+r] / den[h*8+r]) @ Wv.T[:, hb]
    # out = x_cls @ Wp.T + bp
    wvt = np.asarray(Wv, np.float32).T
    wpt = np.asarray(Wp, np.float32).T
    bp = np.asarray(bp, np.float32)
    out = np.empty((B, R, C), np.float32)
    for b in range(B):
        den = res.results[b]["denout"].astype(np.float32).sum(axis=0)  # [HR]
        # slab [p, ct, hr] -> pool [c = ct*128+p, hr]
        poolC = (res.results[b]["slabout"].astype(np.float32)
                 .transpose(1, 0, 2).reshape(C, HR))
        x_cls = np.empty((R, C), np.float32)
        for h in range(H):
            ph = poolC[:, h * R:(h + 1) * R] / den[None, h * R:(h + 1) * R]
            x_cls[:, h * HD:(h + 1) * HD] = ph.T @ wvt[:, h * HD:(h + 1) * HD]
        out[b] = x_cls @ wpt + bp
    return out


if __name__ == "__main__":
    rng = np.random.default_rng(0)
    x = rng.standard_normal((B, N, C)).astype(np.float32)
    mask = rng.integers(0, 2, (B, R, N - R)) > 0
    s = 0.02
    Wq = (rng.standard_normal((C, C)) * s).astype(np.float32)
    Wk = (rng.standard_normal((C, C)) * s).astype(np.float32)
    Wv = (rng.standard_normal((C, C)) * s).astype(np.float32)
    Wp = (rng.standard_normal((C, C)) * s).astype(np.float32)
    bp = np.zeros(C, np.float32)
    out = kernel(x, mask, Wq, Wk, Wv, Wp, bp, 8)
    print("out", out.shape, out.dtype, np.abs(out).mean())


# revision 35
# speedup vs baseline: 1.0468x; 1.0017x over previous
"""AttentionPool kernel for 8x Trainium2 NeuronCores (Bass/Tile).

Problem (per batch b of B=8):
    q = (x[:, :8] @ Wq.T).reshape(8, 24, 64) * 64**-0.5
    k = (x @ Wk.T).reshape(4096, 24, 64)
    v = (x @ Wv.T).reshape(4096, 24, 64)
    attn = softmax(mask(q @ k.T))          # [24, 8, 4096]
    out = (attn @ v).reshape(8, 1536) @ Wp.T + bp

Sharding: data-parallel over B - one batch per NeuronCore, no collectives.

Work split: the device does ALL the O(N) token work; the host does only
O(R*C^2) weight folding (R=8):
  device  logits[n, hr] = x[n, :] . q2[hr, :]   (q2 = q*scale @ Wk folded
          on host), masked exp -> eT, unnormalized pool
          p[c, hr] = sum_n x[n, c] e[n, hr], partial denominators.
  host    q2 fold (76 MFLOP), then the weight epilogue in exact fp32:
          x_cls[r, hb] = (p[:, h*8+r]/den) @ Wv.T[:, hb], out = x_cls @ Wp.T
          + bp (0.9 GFLOP of the 312 GFLOP problem). Wv/Wp never ship to
          the device at all.

Precision/DMA budget (per core, 360 GB/s DMA model, 2.4 GHz PE):
  xt   e4m3  6.29 MB  logits GEMM in DoubleRow perf mode (2 c-tiles per
                      instruction at 0.5 cyc/row): 30.7us -> 15.4us even
                      with the q2 hi/lo residual double-pass.
  q2   e4m3 hi+lo, scaled by 256 (avoids e4m3 subnormal flush at
                      |q2|~0.02), descaled inside the Act exp (scale=1/256).
                      The lo pass cancels the q2 quantization error.
  xn   e3m4  6.29 MB  pool GEMM in normal mode, f16 moving eT (e4m3 xn
                      would cost 1.6e-2 of the 2e-2 budget vs 0.8e-2).
  out  slab [128, 12, 192] f16 + den128 [128, 192] f16 (0.64 MB).
  Host-emulated end-to-end rel err: 1.63e-2 (gate 2e-2, deterministic).

  DMA total 13.9 MB ~= 38.5us; PE = 15.4 (logits DR) + 30.7 (pool) ~= 46us
  is the critical path. All host arrays are partition-major so every
  transfer is >=512B contiguous runs. Stream order: xt0a, q2hi, q2lo,
  xt0b, mask, then xt one chunk AHEAD of xn (xt1, xn0, xt2, xn1, ...) so
  the in-order PE (logits nt, then pool nt-1) never waits on the stream.

Schedule:
  a memset-fed warmup matmul anchors the PE p-state ramp at ~0.5us so all
  real matmuls are charged at full clock (ramp model: 3us to 2.4 GHz).
  per 512-token chunk: logits per 128-token subtile (12 DoubleRow matmuls:
      6 ct-pairs x {q2hi, q2lo}) -> exp (Act, psum -> f32, scale 1/256) ->
      * mask (DVE, broadcast over heads) -> fp16 eT. Pool and den of the
      PREVIOUS chunk are emitted after this chunk's logits so the PE never
      stalls on the exp->mask chain: per c-tile psum[c128, 192] over 4
      subtiles (stationary = e3m4 x subtile, moving = eT), drained into
      one fp16 slab; den[1, 192] via DVE reduce.
  tail: den DMA fires as soon as den(7) lands; the pool-7 slab ships in
      two 6-c-tile pieces (second via the idle Act queue) so the last
      piece's issue overlaps the first piece's transfer.
"""

import numpy as np
import ml_dtypes

B, N, C = 8, 4096, 1536
H, HD, R = 24, 64, 8
HR = H * R           # 192 (h, r) pairs, index hr = h*R + r
SCALE = HD ** -0.5
P = 128
CT = C // P          # 12 contraction tiles
CTP = CT // 2        # 6 DoubleRow ct-pairs
NCHUNK = 512
NSUB_CH = NCHUNK // P  # 4 subtiles per chunk
NT = N // NCHUNK     # 8 chunks
NSUB = N // P        # 32 token subtiles total

Q2SCALE = 256.0      # q2 pre-scale (e4m3 subnormal avoidance)

_RUNNER_CACHE = {}


def _build():
    import concourse.mybir as mybir
    import concourse.tile as tile
    from concourse import bacc

    F32 = mybir.dt.float32
    F16 = mybir.dt.float16
    F8E3 = mybir.dt.float8e3
    F8E4 = mybir.dt.float8e4
    MULT = mybir.AluOpType.mult
    SUB = mybir.AluOpType.subtract
    ADD = mybir.AluOpType.add
    EXP = mybir.ActivationFunctionType.Exp
    DR = mybir.MatmulPerfMode.DoubleRow

    nc = bacc.Bacc(None, target_bir_lowering=False)
    # x.T partition-major: [p, nt, half, ct, 256] = x[512nt+256h+n', 128ct+p]
    xt = nc.dram_tensor("xt", [P, NT, 2, CT, NCHUNK // 2], F8E4,
                        kind="ExternalInput")
    # x partition-major: [p, nt, s, c] = x[512nt+128s+p, c]; chunks 0-3
    # ship e3m4 (pool in normal mode), chunks 4-7 e4m3 (pool in DoubleRow
    # with hi/lo e4m3 eT)
    xn3 = nc.dram_tensor("xn3", [P, NT // 2, NSUB_CH, C], F8E3,
                         kind="ExternalInput")
    xn4 = nc.dram_tensor("xn4", [P, NT // 2, NSUB_CH, C], F8E4,
                         kind="ExternalInput")
    q2hi = nc.dram_tensor("q2hi", [P, CT, HR], F8E4, kind="ExternalInput")
    q2lo = nc.dram_tensor("q2lo", [P, CT, HR], F8E4, kind="ExternalInput")
    # mask partition-major: [p, s, r] = mask_full[r, 128s+p]
    maskt = nc.dram_tensor("maskt", [P, NSUB, R], F16, kind="ExternalInput")
    slabout = nc.dram_tensor("slabout", [P, CT, HR], F16,
                             kind="ExternalOutput")
    denout = nc.dram_tensor("denout", [P, HR], F16, kind="ExternalOutput")

    with tile.TileContext(nc) as tc:
        with (
            tc.tile_pool(name="pper", bufs=1) as pper,      # persistent
            tc.tile_pool(name="pxt", bufs=4) as pxt,        # xT chunks
            tc.tile_pool(name="pxn", bufs=4) as pxn,
            tc.tile_pool(name="pxn4", bufs=4) as pxn4,        # x chunks
            tc.tile_pool(name="pexp", bufs=2) as pexp,
            tc.tile_pool(name="ptmp", bufs=3) as ptmp,
            tc.tile_pool(name="pslab", bufs=1) as pslab,
            tc.tile_pool(name="ps_l", bufs=2, space="PSUM") as ps_l,
            tc.tile_pool(name="ps_p", bufs=6, space="PSUM") as ps_p,
        ):
            # ---------- persistent tiles ----------
            q2hi_sb = pper.tile([P, CT, HR], F8E4, tag="q2hi")
            q2lo_sb = pper.tile([P, CT, HR], F8E4, tag="q2lo")
            maskt_sb = pper.tile([P, NSUB, R], F16, tag="maskt")
            eT = pper.tile([P, NSUB, HR], F16, tag="eT")        # masked exp
            # e4m3 hi/lo split of eT for the DoubleRow pool chunks (si>=16)
            eH = pper.tile([P, NSUB // 2, HR], F8E4, tag="eH")
            eL = pper.tile([P, NSUB // 2, HR], F8E4, tag="eL")
            den128 = pper.tile([P, HR], F16, tag="den128")
            warm = pper.tile([P, 8], F16, tag="warm")

            # ---------- DMA emission helpers (order == queue order) -------
            # xt chunk tile: [p, half, ct, 256]; subtile s of the chunk is
            # [:, s // 2, :, (s % 2) * P:(s % 2 + 1) * P]
            xt_ch0 = pxt.tile([P, 2, CT, NCHUNK // 2], F8E4, tag="xt")

            def _xt_chunk(xt_ch, nt):
                nc.sync.dma_start(xt_ch, xt[:, nt])

            xn_ch0 = pxn.tile([P, NSUB_CH, C], F8E3, tag="xn")

            def _xn_chunk(xn_ch, nt):
                if nt < NT // 2:
                    nc.sync.dma_start(xn_ch, xn3[:, nt])
                else:
                    nc.sync.dma_start(xn_ch, xn4[:, nt - NT // 2])

            # warmup: memset-fed tiny matmuls anchor the PE p-state ramp at
            # ~0.5us, long before the first data-gated matmul, so all real
            # matmuls run at full clock (the ramp model needs 3us of busy
            # history to reach 2.4 GHz)
            nc.vector.memset(warm, 1.0)
            for w in range(3):
                ps_w = ps_l.tile([P, 512], F32, tag="pl")
                nc.tensor.matmul(
                    ps_w[0:1, 0:1], warm[:, w:w + 1], warm[:, w:w + 1],
                    start=True, stop=True)

            # startup stream, ordered by need-time
            nc.sync.dma_start(xt_ch0[:, 0], xt[:, 0, 0])
            nc.sync.dma_start(q2hi_sb, q2hi[:])
            nc.sync.dma_start(q2lo_sb, q2lo[:])
            nc.sync.dma_start(xt_ch0[:, 1], xt[:, 0, 1])
            nc.sync.dma_start(maskt_sb, maskt[:])

            # ---------- per-chunk pipeline ----------
            def emit_logits(nt, xt_ch):
                for s in range(NSUB_CH):
                    si = nt * NSUB_CH + s
                    ps = ps_l.tile([P, 512], F32, tag="pl")
                    lT = ps[:, 0:HR]
                    xsub = xt_ch[:, s // 2, :, (s % 2) * P:(s % 2 + 1) * P]
                    for qi, q2sb in enumerate((q2hi_sb, q2lo_sb)):
                        for t in range(CTP):
                            nc.tensor.matmul(
                                lT,
                                xsub[:, 2 * t:2 * t + 2],
                                q2sb[:, 2 * t:2 * t + 2],
                                start=(qi == 0 and t == 0),
                                stop=(qi == 1 and t == CTP - 1),
                                perf_mode=DR)
                    # exp descales the q2 pre-scale: e = exp(psum / 256)
                    nc.scalar.activation(eT[:, si], lT, EXP, scale=1.0 / Q2SCALE)
                    # in-place 0/1 mask: all operands fp16+SBUF, so the DVE
                    # runs in 2x mode; product is exact (mask is 0 or 1)
                    nc.vector.tensor_tensor(
                        eT[:, si].rearrange("p (h r) -> p h r", h=H),
                        eT[:, si].rearrange("p (h r) -> p h r", h=H),
                        maskt_sb[:, si, None, :].to_broadcast((P, H, R)),
                        MULT)
                    if si >= NSUB // 2:
                        # hi/lo e4m3 split for the DoubleRow pool: hi on the
                        # Act engine, residual lo = eT - hi on the DVE
                        sj = si - NSUB // 2
                        nc.scalar.copy(eH[:, sj], eT[:, si])
                        nc.vector.tensor_tensor(
                            eL[:, sj], eT[:, si], eH[:, sj], SUB)

            def emit_pool(nt, xn_ch, slab, ct_lo=0, ct_hi=CT):
                # pool psum per c-tile; slab accumulates chunks in fp16
                # (adds cost ~5e-4 relative - fine). First chunk drains as
                # copies split across DVE and Act; later chunks add on DVE.
                dr = nt >= NT // 2
                for ct in range(ct_lo, ct_hi):
                    ps = ps_p.tile([P, 512], F32, tag="pp")
                    pch = ps[:, 0:HR]
                    if dr:
                        # DoubleRow: k-tile pairs are subtile pairs; two
                        # passes (eH, eL) cancel the e4m3 eT quantization
                        sj0 = (nt - NT // 2) * NSUB_CH
                        for ei, esb in enumerate((eH, eL)):
                            for u in range(NSUB_CH // 2):
                                nc.tensor.matmul(
                                    pch,
                                    xn_ch[:, 2 * u:2 * u + 2,
                                          ct * P:(ct + 1) * P],
                                    esb[:, sj0 + 2 * u:sj0 + 2 * u + 2],
                                    start=(ei == 0 and u == 0),
                                    stop=(ei == 1 and u == NSUB_CH // 2 - 1),
                                    perf_mode=DR)
                    else:
                        for s in range(NSUB_CH):
                            si = nt * NSUB_CH + s
                            nc.tensor.matmul(
                                pch,
                                xn_ch[:, s, ct * P:(ct + 1) * P],
                                eT[:, si],
                                start=(s == 0), stop=(s == NSUB_CH - 1))
                    # drains split across engines so the DVE isn't the
                    # pacer: even cts add on DVE; odd cts route PSUM->SBUF
                    # through an Act copy (GPSIMD can't read PSUM) and add
                    # SBUF->SBUF on the idle GPSIMD
                    if nt == 0:
                        if ct % 2 == 0:
                            nc.vector.tensor_copy(slab[:, ct], pch)
                        else:
                            nc.scalar.copy(slab[:, ct], pch)
                    elif ct % 2 == 0:
                        nc.vector.tensor_add(slab[:, ct], slab[:, ct], pch)
                    else:
                        tmp = ptmp.tile([P, HR], F16, tag="ptmp",
                                        name=f"tmp_{nt}_{ct}")
                        nc.scalar.copy(tmp, pch)
                        nc.gpsimd.tensor_add(slab[:, ct], slab[:, ct], tmp)

            def emit_den(nt):
                # per-partition partial denominators on the DVE (idle
                # capacity): innermost-axis reduce on a strided view; the
                # cross-partition sum happens on the host in fp32.
                sl = eT[:, nt * NSUB_CH:(nt + 1) * NSUB_CH].rearrange(
                    "p s h -> p h s")
                with nc.allow_low_precision(reason="den rel err ~5e-4"):
                    if nt == 0:
                        nc.vector.tensor_reduce(
                            den128, sl, mybir.AxisListType.X, ADD)
                    else:
                        dpart = pexp.tile([P, HR], F16, tag="dpart")
                        nc.vector.tensor_reduce(
                            dpart, sl, mybir.AxisListType.X, ADD)
                        nc.vector.tensor_add(den128, den128, dpart)

            slab = pslab.tile([P, CT, HR], F16, tag="slab", name="slab")
            xns = []
            xt_ch = xt_ch0
            for nt in range(NT):
                # stream xt one chunk AHEAD of xn: the PE runs logits(nt)
                # then pool(nt-1), so it needs xt(nt) before xn(nt-1)
                if nt + 1 < NT:
                    xt_nx = pxt.tile([P, 2, CT, NCHUNK // 2], F8E4, tag="xt")
                    _xt_chunk(xt_nx, nt + 1)
                else:
                    xt_nx = None
                if nt >= NT // 2:
                    xn_ch = pxn4.tile([P, NSUB_CH, C], F8E4, tag="xn4",
                                      name=f"xn_ch{nt}")
                    _xn_chunk(xn_ch, nt)
                elif nt > 0:
                    xn_ch = pxn.tile([P, NSUB_CH, C], F8E3, tag="xn",
                                     name=f"xn_ch{nt}")
                    _xn_chunk(xn_ch, nt)
                else:
                    # chunk 0's xn loads in halves so pool-0 (the head of
                    # the PE's steady pipeline) starts a hop earlier
                    xn_ch = xn_ch0
                    nc.sync.dma_start(xn_ch0[:, 0:2], xn3[:, 0, 0:2])
                    nc.sync.dma_start(xn_ch0[:, 2:4], xn3[:, 0, 2:4])
                emit_logits(nt, xt_ch)
                if nt > 0:
                    emit_pool(nt - 1, xns[nt - 1], slab)
                    emit_den(nt - 1)
                xns.append(xn_ch)
                xt_ch = xt_nx

            # ---------- tail ----------
            # den(7) completes on the DVE as soon as eT(7) is masked; its
            # DMA fires while the pool-7 matmuls still run. The slab ships
            # in two 6-c-tile pieces; the second goes through the idle Act
            # queue so both issue chains overlap.
            emit_den(NT - 1)
            nc.sync.dma_start(denout[:], den128)
            emit_pool(NT - 1, xns[NT - 1], slab, 0, 6)
            nc.sync.dma_start(slabout[:, 0:6], slab[:, 0:6])
            emit_pool(NT - 1, xns[NT - 1], slab, 6, CT)
            nc.sync.dma_start(slabout[:, 6:CT], slab[:, 6:CT])

    nc.compile()
    return nc


def _prep_inputs(x, mask, Wq, Wk):
    """Host-side sharding + layout prep. Returns per-core in_maps.

    The 8-token q projection and its fold through Wk (q2 = q*scale @
    Wk[head rows]) happen here: 76 MFLOP of the 312 GFLOP problem."""
    E3 = ml_dtypes.float8_e3m4
    E4 = ml_dtypes.float8_e4m3

    x = np.asarray(x, dtype=np.float32)
    Wq = np.asarray(Wq, np.float32)
    Wk = np.asarray(Wk, np.float32)

    mask = np.asarray(mask)
    mask_full = np.empty((B, R, N), np.float32)
    mask_full[:, :, :R] = np.eye(R, dtype=np.float32)[None]
    mask_full[:, :, R:] = mask.astype(np.float32)

    # q2[b, hr, c] = sum_d q[b, r, h, d]*SCALE * Wk[h*HD+d, c]
    q = np.einsum('brc,dc->brd', x[:, :R], Wq) * SCALE        # [B, R, C]
    q2 = np.einsum('brhd,hdc->bhrc',
                   q.reshape(B, R, H, HD), Wk.reshape(H, HD, C))
    q2 = q2.reshape(B, HR, C)

    in_maps = []
    for b in range(B):
        xb = np.clip(x[b], -15.0, 15.0)
        # xt: [p, nt, half, ct, 256] = x[512nt+256h+n', 128ct+p], e4m3
        xt_b = np.ascontiguousarray(
            xb.T.reshape(CT, P, NT, 2, NCHUNK // 2)
            .transpose(1, 2, 3, 0, 4).astype(E4))
        # xn: [p, nt, s, c] = x[512nt+128s+p, c]; chunks 0-3 e3m4
        # (normal-mode pool), chunks 4-7 e4m3 (DoubleRow pool)
        xn_all = xb.reshape(NT, NSUB_CH, P, C).transpose(2, 0, 1, 3)
        xn3_b = np.ascontiguousarray(xn_all[:, :NT // 2].astype(E3))
        xn4_b = np.ascontiguousarray(xn_all[:, NT // 2:].astype(E4))
        # q2 hi/lo, scaled by 256, e4m3, [p, ct, hr]
        q2s = q2[b].T * Q2SCALE                               # [C, HR]
        q2hi = q2s.astype(E4).astype(np.float32)
        q2lo = (q2s - q2hi).astype(E4)
        q2hi_b = np.ascontiguousarray(
            q2hi.astype(E4).reshape(CT, P, HR).transpose(1, 0, 2))
        q2lo_b = np.ascontiguousarray(
            q2lo.reshape(CT, P, HR).transpose(1, 0, 2))
        # mask: [p, s, r] = mask_full[r, 128s+p]
        maskt_b = np.ascontiguousarray(
            mask_full[b].T.reshape(NSUB, P, R).transpose(1, 0, 2)
            .astype(np.float16))
        in_maps.append({
            "xt": xt_b, "xn3": xn3_b, "xn4": xn4_b,
            "q2hi": q2hi_b, "q2lo": q2lo_b, "maskt": maskt_b,
        })
    return in_maps


def _get_nc():
    if "nc" not in _RUNNER_CACHE:
        _RUNNER_CACHE["nc"] = _build()
    return _RUNNER_CACHE["nc"]


def kernel(x, mask, Wq, Wk, Wv, Wp, bp, repeats=8, **_unused):
    from concourse import bass_utils

    in_maps = _prep_inputs(x, mask, Wq, Wk)
    nc = _get_nc()
    res = bass_utils.run_bass_kernel_spmd(nc, in_maps, core_ids=list(range(B)))

    # host weight epilogue in exact fp32 (O(R*C^2), no O(N) work):
    # x_cls[r, hb] = (p[:, h*8+r] / den[h*8+r]) @ Wv.T[:, hb]
    # out = x_cls @ Wp.T + bp
    wvt = np.asarray(Wv, np.float32).T
    wpt = np.asarray(Wp, np.float32).T
    bp = np.asarray(bp, np.float32)
    out = np.empty((B, R, C), np.float32)
    for b in range(B):
        den = res.results[b]["denout"].astype(np.float32).sum(axis=0)  # [HR]
        # slab [p, ct, hr] -> pool [c = ct*128+p, hr]
        poolC = (res.results[b]["slabout"].astype(np.float32)
                 .transpose(1, 0, 2).reshape(C, HR))
        x_cls = np.empty((R, C), np.float32)
        for h in range(H):
            ph = poolC[:, h * R:(h + 1) * R] / den[None, h * R:(h + 1) * R]
            x_cls[:, h * HD:(h + 1) * HD] = ph.T @ wvt[:, h * HD:(h + 1) * HD]
        out[b] = x_cls @ wpt + bp
    return out


if __name__ == "__main__":
    rng = np.random.default_rng(0)
    x = rng.standard_normal((B, N, C)).astype(np.float32)
    mask = rng.integers(0, 2, (B, R, N - R)) > 0
    s = 0.02
    Wq = (rng.standard_normal((C, C)) * s).astype(np.float32)
    Wk = (rng.standard_normal((C, C)) * s).astype(np.float32)
    Wv = (rng.standard_normal((C, C)) * s).astype(np.float32)
    Wp = (rng.standard_normal((C, C)) * s).astype(np.float32)
    bp = np.zeros(C, np.float32)
    out = kernel(x, mask, Wq, Wk, Wv, Wp, bp, 8)
    print("out", out.shape, out.dtype, np.abs(out).mean())
